# revision 4
# baseline (speedup 1.0000x reference)
"""Single-head full-attention layer on 8 Trainium2 NeuronCores.

reference:
    q = seq @ Wq; k = seq @ Wk; v = seq @ Wv          # [B,S,D], D=1024
    scores = q @ k.T / sqrt(D)                        # [B,S,S]
    out = seq + softmax(scores) @ v * mask            # [B,S,D]

Sharding: 8 cores = 4 batches x 2 sequence-halves. Each core:
  - computes Q for its own 1024 queries,
  - computes K^T/V for its own 1024 keys only,
  - exchanges K^T/V halves with its pair partner via 2-core AllGathers
    (K exchanged early so its latency hides under the Q projection),
  - runs softmax(QK^T)V + mask + residual for its query half.

All tensors are kept transposed on device ([d] or [key] on partitions,
queries on the free axis) so every matmul is a natural lhsT.T @ rhs:

    KT_own = Wk-chunks.T @ seqT_own   [d, key_own]   -> AllGather -> KT
    V_own  = seqT-chunks.T @ Wv       [key_own, d]   -> AllGather -> V
    QT     = Wq-chunks.T @ seqT_own   [d, q]
    ST     = KT-chunks.T @ QT         [key, q]
    AT     = exp(ST / 32)             bf16, unnormalized (scores ~ N(0,1))
    colsum[q] = ones.T @ AT           (PE reduction over keys)
    OT     = V-chunks.T @ AT          [d, q]
    out    = OT * (1/colsum) + seqT_half_f32   (mask folded into Wv on host)

The SPMD program is identical on all cores: the host feeds each core the
transposed bf16 slice of seq for its own half; the AllGather delivers
keys in global order for everyone.
"""

import numpy as np
import ml_dtypes

import concourse.bass as bass
import concourse.mybir as mybir
import concourse.tile as tile
from concourse import bacc, bass_utils

B, S, D = 4, 2048, 1024
N_CORES = 8
SH = S // 2          # queries / own keys per core
PD = 128             # partition dim
KD = D // PD         # 8 chunks over d
KH = SH // PD        # 8 chunks over own keys
KC = S // PD         # 16 chunks over all keys
NT = 512             # matmul free-dim tile (one PSUM bank of fp32)
BF = mybir.dt.bfloat16
F32 = mybir.dt.float32
EXP_SCALE = 1.0 / 32.0   # 1/sqrt(D)

_BF16 = ml_dtypes.bfloat16


def _build_kernel(tc):
    nc = tc.nc
    seqT = nc.dram_tensor("seqT", [D, SH], BF, kind="ExternalInput").ap()
    wq = nc.dram_tensor("wq", [D, D], BF, kind="ExternalInput").ap()
    wk = nc.dram_tensor("wk", [D, D], BF, kind="ExternalInput").ap()
    wv = nc.dram_tensor("wv", [D, D], BF, kind="ExternalInput").ap()
    seqTh = nc.dram_tensor("seqTh", [D, SH], F32, kind="ExternalInput").ap()
    outT = nc.dram_tensor("outT", [D, SH], F32, kind="ExternalOutput").ap()

    Exp = mybir.ActivationFunctionType.Exp

    with (
        tc.tile_pool(name="p_seq", bufs=1) as p_seq,
        tc.tile_pool(name="p_w", bufs=2) as p_w,
        tc.tile_pool(name="p_own", bufs=1) as p_own,
        tc.tile_pool(name="p_qt", bufs=1) as p_qt,
        tc.tile_pool(name="p_kt", bufs=1) as p_kt,
        tc.tile_pool(name="p_v", bufs=1) as p_v,
        tc.tile_pool(name="p_at", bufs=1) as p_at,
        tc.tile_pool(name="p_sh", bufs=2) as p_sh,
        tc.tile_pool(name="p_o", bufs=2) as p_o,
        tc.tile_pool(name="p_msc", bufs=1) as p_msc,
        tc.tile_pool(name="p_dram", bufs=1, space="DRAM") as p_dram,
        tc.tile_pool(name="p_mm", bufs=6, space="PSUM") as p_mm,
        tc.tile_pool(name="p_cs", bufs=1, space="PSUM") as p_cs,
    ):
        # ---- resident inputs ------------------------------------------------
        seq_sb = []
        for i in range(KD):
            t = p_seq.tile([PD, SH], BF, tag=f"s{i}", name=f"seq{i}")
            nc.sync.dma_start(t[:], seqT[i * PD:(i + 1) * PD, :])
            seq_sb.append(t)

        def load_w(w_dram, label):
            chunks = []
            for i in range(KD):
                t = p_w.tile([PD, D], BF, tag=f"w{i}", name=f"{label}{i}")
                nc.sync.dma_start(t[:], w_dram[i * PD:(i + 1) * PD, :])
                chunks.append(t)
            return chunks

        # collective bounce buffers (DRAM, Local)
        ib_kt = p_dram.tile([D, SH], BF, tag="ibk", name="ib_kt")
        ob_kt = p_dram.tile([2, D, SH], BF, tag="obk", name="ob_kt")
        ib_v = p_dram.tile([SH, D], BF, tag="ibv", name="ib_v")
        ob_v = p_dram.tile([2, SH, D], BF, tag="obv", name="ob_v")

        # tiny warmup AllGather: absorbs the one-time cc-stream init +
        # barrier (~12us + ~11us trigger delay) during the startup DMAs,
        # so the real KT exchange starts immediately when triggered.
        ib_wu = p_dram.tile([1, 16], BF, tag="ibw", name="ib_wu")
        ob_wu = p_dram.tile([2, 16], BF, tag="obw", name="ob_wu")
        nc.gpsimd.collective_compute(
            "AllGather",
            mybir.AluOpType.bypass,
            replica_groups=[[0, 1], [2, 3], [4, 5], [6, 7]],
            ins=[ib_wu.opt()],
            outs=[ob_wu.opt()],
        )

        # ---- KT_own = (seq_own @ Wk).T, bounce out, AllGather ---------------
        wk_sb = load_w(wk, "wk")
        kto_sb = []
        for m in range(KD):
            t = p_own.tile([PD, SH], BF, tag=f"x{m}", name=f"kto{m}")
            kto_sb.append(t)
        for m in range(KD):
            nn = SH // NT
            pss = [p_mm.tile([PD, NT], F32, tag="mm", name=f"ps_k{m}_{n}")
                   for n in range(nn)]
            for k in range(KD):
                for n in range(nn):
                    nc.tensor.matmul(
                        pss[n][:],
                        wk_sb[k][:, m * PD:(m + 1) * PD],
                        seq_sb[k][:, n * NT:(n + 1) * NT],
                        start=(k == 0),
                        stop=(k == KD - 1),
                    )
            for n in range(nn):
                nc.vector.tensor_copy(kto_sb[m][:, n * NT:(n + 1) * NT], pss[n][:])
            nc.sync.dma_start(ib_kt[m * PD:(m + 1) * PD, :], kto_sb[m][:])
        nc.gpsimd.collective_compute(
            "AllGather",
            mybir.AluOpType.bypass,
            replica_groups=[[0, 1], [2, 3], [4, 5], [6, 7]],
            ins=[ib_kt.opt()],
            outs=[ob_kt.opt()],
        )

        # ---- V_own = seq_own @ (Wv * mask), bounce out, AllGather -----------
        wv_sb = load_w(wv, "wv")
        vo_sb = []
        for m in range(KH):
            t = p_own.tile([PD, D], BF, tag=f"x{m}", name=f"vo{m}")
            vo_sb.append(t)
        for m in range(KH):
            nn = D // NT
            pss = [p_mm.tile([PD, NT], F32, tag="mm", name=f"ps_v{m}_{n}")
                   for n in range(nn)]
            for k in range(KD):
                for n in range(nn):
                    nc.tensor.matmul(
                        pss[n][:],
                        seq_sb[k][:, m * PD:(m + 1) * PD],
                        wv_sb[k][:, n * NT:(n + 1) * NT],
                        start=(k == 0),
                        stop=(k == KD - 1),
                    )
            for n in range(nn):
                nc.vector.tensor_copy(vo_sb[m][:, n * NT:(n + 1) * NT], pss[n][:])
            nc.sync.dma_start(ib_v[m * PD:(m + 1) * PD, :], vo_sb[m][:])
        nc.gpsimd.collective_compute(
            "AllGather",
            mybir.AluOpType.bypass,
            replica_groups=[[0, 1], [2, 3], [4, 5], [6, 7]],
            ins=[ib_v.opt()],
            outs=[ob_v.opt()],
        )

        # ---- QT = (seq_own @ Wq).T (overlaps the collectives) ---------------
        wq_sb = load_w(wq, "wq")
        qt_sb = [p_qt.tile([PD, SH], BF, tag=f"q{m}", name=f"qt{m}") for m in range(KD)]
        for m in range(KD):
            nn = SH // NT
            pss = [p_mm.tile([PD, NT], F32, tag="mm", name=f"ps_q{m}_{n}")
                   for n in range(nn)]
            for k in range(KD):
                for n in range(nn):
                    nc.tensor.matmul(
                        pss[n][:],
                        wq_sb[k][:, m * PD:(m + 1) * PD],
                        seq_sb[k][:, n * NT:(n + 1) * NT],
                        start=(k == 0),
                        stop=(k == KD - 1),
                    )
            for n in range(nn):
                nc.vector.tensor_copy(qt_sb[m][:, n * NT:(n + 1) * NT], pss[n][:])

        # ---- gather exchanged KT / V into SBUF ------------------------------
        kt_sb = [p_kt.tile([PD, S], BF, tag=f"k{m}", name=f"kt{m}") for m in range(KD)]
        for m in range(KD):
            for r in range(2):
                nc.sync.dma_start(
                    kt_sb[m][:, r * SH:(r + 1) * SH],
                    ob_kt[r, m * PD:(m + 1) * PD, :],
                )
        v_sb = [p_v.tile([PD, D], BF, tag=f"v{m}", name=f"v{m}") for m in range(KC)]
        for m in range(KC):
            r, mm_ = divmod(m, KH)
            nc.sync.dma_start(v_sb[m][:], ob_v[r, mm_ * PD:(mm_ + 1) * PD, :])

        # ---- scoresT -> exp -> colsum ---------------------------------------
        ones_sb = p_msc.tile([PD, 1], BF, tag="ones", name="ones")
        nc.vector.memset(ones_sb[:], 1.0)
        cs_ps = p_cs.tile([1, SH], F32, tag="cs", name="cs")
        at_sb = [p_at.tile([PD, SH], BF, tag=f"a{m}", name=f"at{m}") for m in range(KC)]

        def colsum_mm(m):
            for n in range(SH // NT):
                nc.tensor.matmul(
                    cs_ps[:, n * NT:(n + 1) * NT],
                    ones_sb[:],
                    at_sb[m][:, n * NT:(n + 1) * NT],
                    start=(m == 0),
                    stop=(m == KC - 1),
                )

        for m in range(KC):
            nn = SH // NT
            pss = [p_mm.tile([PD, NT], F32, tag="mm", name=f"ps_s{m}_{n}")
                   for n in range(nn)]
            for k in range(KD):
                for n in range(nn):
                    nc.tensor.matmul(
                        pss[n][:],
                        kt_sb[k][:, m * PD:(m + 1) * PD],
                        qt_sb[k][:, n * NT:(n + 1) * NT],
                        start=(k == 0),
                        stop=(k == KD - 1),
                    )
            for n in range(nn):
                nc.scalar.activation(
                    at_sb[m][:, n * NT:(n + 1) * NT], pss[n][:], Exp, scale=EXP_SCALE
                )
            # one chunk late so the PE never waits on ACT's exp
            if m > 0:
                colsum_mm(m - 1)
        colsum_mm(KC - 1)

        # ---- 1/colsum, broadcast across partitions --------------------------
        recip_sb = p_msc.tile([1, SH], F32, tag="recip", name="recip")
        nc.vector.reciprocal(recip_sb[:], cs_ps[:])
        bc_sb = p_msc.tile([PD, SH], F32, tag="bc", name="bc")
        nc.gpsimd.partition_broadcast(bc_sb[:], recip_sb[:])

        # ---- OT = V.T @ AT, normalize, add residual, store ------------------
        for m in range(KD):
            sh_t = p_sh.tile([PD, SH], F32, tag="sh", name=f"sh{m}")
            nc.sync.dma_start(sh_t[:], seqTh[m * PD:(m + 1) * PD, :])
            o_t = p_o.tile([PD, SH], F32, tag="o", name=f"o{m}")
            nn = SH // NT
            pss = [p_mm.tile([PD, NT], F32, tag="mm", name=f"ps_o{m}_{n}")
                   for n in range(nn)]
            for k in range(KC):
                for n in range(nn):
                    nc.tensor.matmul(
                        pss[n][:],
                        v_sb[k][:, m * PD:(m + 1) * PD],
                        at_sb[k][:, n * NT:(n + 1) * NT],
                        start=(k == 0),
                        stop=(k == KC - 1),
                    )
            for n in range(nn):
                nc.vector.tensor_mul(
                    o_t[:, n * NT:(n + 1) * NT], pss[n][:],
                    bc_sb[:, n * NT:(n + 1) * NT]
                )
            nc.vector.tensor_add(o_t[:], o_t[:], sh_t[:])
            nc.sync.dma_start(outT[m * PD:(m + 1) * PD, :], o_t[:])


_NC_CACHE = None


def _get_nc():
    global _NC_CACHE
    if _NC_CACHE is None:
        nc = bacc.Bacc(
            "TRN2", target_bir_lowering=False, debug=False, num_devices=N_CORES
        )
        with tile.TileContext(nc) as tc:
            _build_kernel(tc)
        nc.compile()
        _NC_CACHE = nc
    return _NC_CACHE


def _prep_in_maps(seq, Wq, Wk, Wv, mask):
    seq = np.asarray(seq, dtype=np.float32)
    wq_bf = np.asarray(Wq, dtype=np.float32).astype(_BF16)
    wk_bf = np.asarray(Wk, dtype=np.float32).astype(_BF16)
    wvm_bf = (np.asarray(Wv, dtype=np.float32)
              * np.asarray(mask, dtype=np.float32)[None, :]).astype(_BF16)
    in_maps = []
    for c in range(N_CORES):
        b, h = divmod(c, 2)
        seqT_own = np.ascontiguousarray(seq[b, h * SH:(h + 1) * SH, :].T)  # [D, SH]
        in_maps.append({
            "seqT": seqT_own.astype(_BF16),
            "wq": wq_bf,
            "wk": wk_bf,
            "wv": wvm_bf,
            "seqTh": seqT_own,
        })
    return in_maps


def _run(seq, Wq, Wk, Wv, mask, trace=False, **run_kwargs):
    nc = _get_nc()
    in_maps = _prep_in_maps(seq, Wq, Wk, Wv, mask)
    res = bass_utils.run_bass_kernel_spmd(
        nc, in_maps, core_ids=list(range(N_CORES)), trace=trace, **run_kwargs
    )
    out = np.empty((B, S, D), dtype=np.float32)
    for c in range(N_CORES):
        b, h = divmod(c, 2)
        out[b, h * SH:(h + 1) * SH, :] = res.results[c]["outT"].T
    return out, res


def kernel(seq, Wq, Wk, Wv, mask):
    out, _ = _run(seq, Wq, Wk, Wv, mask)
    return out


# revision 8
# speedup vs baseline: 1.5581x; 1.5581x over previous
"""Single-head full-attention layer on 8 Trainium2 NeuronCores (fp8 DoubleRow).

reference:
    q = seq @ Wq; k = seq @ Wk; v = seq @ Wv          # [B,S,D], D=1024
    scores = q @ k.T / sqrt(D)                        # [B,S,S]
    out = seq + softmax(scores) @ v * mask            # [B,S,D]

Sharding: 8 cores = 4 batches x 2 sequence-halves. Each core computes Q
for its own 1024 queries and K^T/V for its own 1024 keys; pairs exchange
K^T/V halves via 2-core AllGathers; each core then runs
softmax(QK^T)V + mask + residual for its query half.

All matmuls run in fp8(E4M3) with perf_mode=DoubleRow (two contraction
rows per PE cell): operands live in 3D SBUF tiles [128, ksub, free] and
each matmul consumes a [:, k:k+2, :] slice. Numerics:
  - host scales W by 32 before the fp8 cast (keeps N(0,1/1024) weights
    out of the fp8 subnormal range); the q/k factors cancel inside
    exp's scale (2^-15) and V's factor is folded into 1/colsum.
  - exp is shifted by -3 so attn values stay below fp8e4's +-240 max
    (softmax is shift-invariant).
  - scores/colsum/outT accumulate in fp32 PSUM; the normalization,
    output mask (folded into Wv on host) and fp32 residual are applied
    in the final fp32 stage.

The SPMD program is identical on all cores; the AllGather delivers keys
in global order for everyone.
"""

import numpy as np
import ml_dtypes

import concourse.bass as bass
import concourse.mybir as mybir
import concourse.tile as tile
from concourse import bacc, bass_utils

B, S, D = 4, 2048, 1024
N_CORES = 8
SH = S // 2          # queries / own keys per core
PD = 128             # partition dim
KD = D // PD         # 8 ksub chunks over d
KH = SH // PD        # 8 ksub chunks over own keys
KC = S // PD         # 16 ksub chunks over all keys
NT = 512             # matmul free-dim tile (one PSUM bank of fp32)
F8 = mybir.dt.float8e4
F32 = mybir.dt.float32
W_SCALE = 32.0
EXP_SCALE = 1.0 / (32.0 * W_SCALE * W_SCALE)   # 1/sqrt(D) / W_SCALE^2
EXP_SHIFT = -3.0
DR = mybir.MatmulPerfMode.DoubleRow

_FP8 = ml_dtypes.float8_e4m3
_GROUPS = [[0, 1], [2, 3], [4, 5], [6, 7]]


def _build_kernel(tc):
    nc = tc.nc
    seqT = nc.dram_tensor("seqT", [D, SH], F8, kind="ExternalInput").ap()
    wq = nc.dram_tensor("wq", [D, D], F8, kind="ExternalInput").ap()
    wk = nc.dram_tensor("wk", [D, D], F8, kind="ExternalInput").ap()
    wv = nc.dram_tensor("wv", [D, D], F8, kind="ExternalInput").ap()
    seqTh = nc.dram_tensor("seqTh", [D, SH], F32, kind="ExternalInput").ap()
    outT = nc.dram_tensor("outT", [D, SH], F32, kind="ExternalOutput").ap()

    Exp = mybir.ActivationFunctionType.Exp

    with (
        tc.tile_pool(name="p_seq", bufs=1) as p_seq,
        tc.tile_pool(name="p_w", bufs=2) as p_w,
        tc.tile_pool(name="p_own", bufs=1) as p_own,
        tc.tile_pool(name="p_qt", bufs=1) as p_qt,
        tc.tile_pool(name="p_kt", bufs=1) as p_kt,
        tc.tile_pool(name="p_v", bufs=1) as p_v,
        tc.tile_pool(name="p_at", bufs=1) as p_at,
        tc.tile_pool(name="p_sh", bufs=2) as p_sh,
        tc.tile_pool(name="p_o", bufs=2) as p_o,
        tc.tile_pool(name="p_msc", bufs=1) as p_msc,
        tc.tile_pool(name="p_dram", bufs=1, space="DRAM") as p_dram,
        tc.tile_pool(name="p_mm", bufs=6, space="PSUM") as p_mm,
        tc.tile_pool(name="p_cs", bufs=1, space="PSUM") as p_cs,
    ):
        # ---- resident inputs ------------------------------------------------
        seq3 = p_seq.tile([PD, KD, SH], F8, tag="seq", name="seq3")
        for j in range(KD):
            nc.sync.dma_start(seq3[:, j, :], seqT[j * PD:(j + 1) * PD, :])

        def load_w(w_dram, label):
            t = p_w.tile([PD, KD, D], F8, tag="w", name=label)
            for j in range(KD):
                nc.sync.dma_start(t[:, j, :], w_dram[j * PD:(j + 1) * PD, :])
            return t

        # collective bounce buffers (DRAM, Local)
        ib_kt = p_dram.tile([D, SH], F8, tag="ibk", name="ib_kt")
        ob_kt = p_dram.tile([2, D, SH], F8, tag="obk", name="ob_kt")
        ib_v = p_dram.tile([SH, D], F8, tag="ibv", name="ib_v")
        ob_v = p_dram.tile([2, SH, D], F8, tag="obv", name="ob_v")

        # ---- KT_own = (seq_own @ Wk).T, bounce out, AllGather ---------------
        wk3 = load_w(wk, "wk3")
        kto3 = p_own.tile([PD, KD, SH], F8, tag="kto", name="kto3")
        for m in range(KD):
            for n in range(SH // NT):
                ps = p_mm.tile([PD, NT], F32, tag="mm", name=f"ps_k{m}_{n}")
                for k in range(0, KD, 2):
                    nc.tensor.matmul(
                        ps[:],
                        wk3[:, k:k + 2, m * PD:(m + 1) * PD],
                        seq3[:, k:k + 2, n * NT:(n + 1) * NT],
                        start=(k == 0),
                        stop=(k == KD - 2),
                        perf_mode=DR,
                    )
                nc.vector.tensor_copy(kto3[:, m, n * NT:(n + 1) * NT], ps[:])
            nc.sync.dma_start(ib_kt[m * PD:(m + 1) * PD, :], kto3[:, m, :])
        nc.gpsimd.collective_compute(
            "AllGather", mybir.AluOpType.bypass, replica_groups=_GROUPS,
            ins=[ib_kt.opt()], outs=[ob_kt.opt()],
        )

        # ---- V_own = seq_own @ (Wv * mask), bounce out, AllGather -----------
        wv3 = load_w(wv, "wv3")
        vo3 = p_own.tile([PD, KH, D], F8, tag="vo", name="vo3")
        for m in range(KH):
            for n in range(D // NT):
                ps = p_mm.tile([PD, NT], F32, tag="mm", name=f"ps_v{m}_{n}")
                for k in range(0, KD, 2):
                    nc.tensor.matmul(
                        ps[:],
                        seq3[:, k:k + 2, m * PD:(m + 1) * PD],
                        wv3[:, k:k + 2, n * NT:(n + 1) * NT],
                        start=(k == 0),
                        stop=(k == KD - 2),
                        perf_mode=DR,
                    )
                nc.vector.tensor_copy(vo3[:, m, n * NT:(n + 1) * NT], ps[:])
            nc.sync.dma_start(ib_v[m * PD:(m + 1) * PD, :], vo3[:, m, :])
        nc.gpsimd.collective_compute(
            "AllGather", mybir.AluOpType.bypass, replica_groups=_GROUPS,
            ins=[ib_v.opt()], outs=[ob_v.opt()],
        )

        # ---- QT = (seq_own @ Wq).T (overlaps the collectives) ---------------
        wq3 = load_w(wq, "wq3")
        qt3 = p_qt.tile([PD, KD, SH], F8, tag="qt", name="qt3")
        for m in range(KD):
            for n in range(SH // NT):
                ps = p_mm.tile([PD, NT], F32, tag="mm", name=f"ps_q{m}_{n}")
                for k in range(0, KD, 2):
                    nc.tensor.matmul(
                        ps[:],
                        wq3[:, k:k + 2, m * PD:(m + 1) * PD],
                        seq3[:, k:k + 2, n * NT:(n + 1) * NT],
                        start=(k == 0),
                        stop=(k == KD - 2),
                        perf_mode=DR,
                    )
                nc.vector.tensor_copy(qt3[:, m, n * NT:(n + 1) * NT], ps[:])

        # ---- gather exchanged KT / V into SBUF ------------------------------
        kt3 = p_kt.tile([PD, KD, S], F8, tag="kt", name="kt3")
        for m in range(KD):
            for r in range(2):
                nc.sync.dma_start(
                    kt3[:, m, r * SH:(r + 1) * SH],
                    ob_kt[r, m * PD:(m + 1) * PD, :],
                )
        v3t = p_v.tile([PD, KC, D], F8, tag="v", name="v3t")
        for m in range(KC):
            r, mm_ = divmod(m, KH)
            nc.sync.dma_start(v3t[:, m, :], ob_v[r, mm_ * PD:(mm_ + 1) * PD, :])

        # ---- scoresT -> exp(shifted) -> colsum ------------------------------
        ones3 = p_msc.tile([PD, 2, 16], F8, tag="ones", name="ones3")
        nc.vector.memset(ones3[:], 1.0)
        ebias = p_msc.tile([PD, 1], F32, tag="ebias", name="ebias")
        nc.vector.memset(ebias[:], EXP_SHIFT)
        cs_ps = p_cs.tile([1, SH], F32, tag="cs", name="cs")
        at3 = p_at.tile([PD, KC, SH], F8, tag="at", name="at3")

        def colsum_mm(m):
            for n in range(SH // NT):
                nc.tensor.matmul(
                    cs_ps[:, n * NT:(n + 1) * NT],
                    ones3[:, 0:2, 0:1],
                    at3[:, m:m + 2, n * NT:(n + 1) * NT],
                    start=(m == 0),
                    stop=(m == KC - 2),
                    perf_mode=DR,
                )

        for m in range(KC):
            for n in range(SH // NT):
                ps = p_mm.tile([PD, NT], F32, tag="mm", name=f"ps_s{m}_{n}")
                for k in range(0, KD, 2):
                    nc.tensor.matmul(
                        ps[:],
                        kt3[:, k:k + 2, m * PD:(m + 1) * PD],
                        qt3[:, k:k + 2, n * NT:(n + 1) * NT],
                        start=(k == 0),
                        stop=(k == KD - 2),
                        perf_mode=DR,
                    )
                nc.scalar.activation(
                    at3[:, m, n * NT:(n + 1) * NT], ps[:], Exp,
                    bias=ebias[:], scale=EXP_SCALE,
                )
            # colsum pairs ksubs (m, m+1); emit one pair late so the PE
            # never waits on ACT's exp
            if m >= 3 and m % 2 == 1:
                colsum_mm(m - 3)
        colsum_mm(KC - 2)

        # ---- 1/colsum (including V's W_SCALE), broadcast across partitions --
        recip_sb = p_msc.tile([1, SH], F32, tag="recip", name="recip")
        nc.vector.reciprocal(recip_sb[:], cs_ps[:])
        nc.vector.tensor_scalar_mul(recip_sb[:], recip_sb[:], 1.0 / W_SCALE)
        bc_sb = p_msc.tile([PD, SH], F32, tag="bc", name="bc")
        nc.gpsimd.partition_broadcast(bc_sb[:], recip_sb[:])

        # ---- OT = V.T @ AT, normalize, add residual, store ------------------
        for m in range(KD):
            sh_t = p_sh.tile([PD, SH], F32, tag="sh", name=f"sh{m}")
            nc.sync.dma_start(sh_t[:], seqTh[m * PD:(m + 1) * PD, :])
            o_t = p_o.tile([PD, SH], F32, tag="o", name=f"o{m}")
            for n in range(SH // NT):
                ps = p_mm.tile([PD, NT], F32, tag="mm", name=f"ps_o{m}_{n}")
                for k in range(0, KC, 2):
                    nc.tensor.matmul(
                        ps[:],
                        v3t[:, k:k + 2, m * PD:(m + 1) * PD],
                        at3[:, k:k + 2, n * NT:(n + 1) * NT],
                        start=(k == 0),
                        stop=(k == KC - 2),
                        perf_mode=DR,
                    )
                nc.vector.tensor_mul(
                    o_t[:, n * NT:(n + 1) * NT], ps[:],
                    bc_sb[:, n * NT:(n + 1) * NT],
                )
            nc.vector.tensor_add(o_t[:], o_t[:], sh_t[:])
            nc.sync.dma_start(outT[m * PD:(m + 1) * PD, :], o_t[:])


_NC_CACHE = None


def _get_nc():
    global _NC_CACHE
    if _NC_CACHE is None:
        nc = bacc.Bacc(
            "TRN2", target_bir_lowering=False, debug=False, num_devices=N_CORES
        )
        with tile.TileContext(nc) as tc:
            _build_kernel(tc)
        nc.compile()
        _NC_CACHE = nc
    return _NC_CACHE


def _prep_in_maps(seq, Wq, Wk, Wv, mask):
    seq = np.asarray(seq, dtype=np.float32)
    wq_f8 = (np.asarray(Wq, dtype=np.float32) * W_SCALE).astype(_FP8)
    wk_f8 = (np.asarray(Wk, dtype=np.float32) * W_SCALE).astype(_FP8)
    wvm_f8 = (np.asarray(Wv, dtype=np.float32)
              * np.asarray(mask, dtype=np.float32)[None, :] * W_SCALE).astype(_FP8)
    in_maps = []
    for c in range(N_CORES):
        b, h = divmod(c, 2)
        seqT_own = np.ascontiguousarray(seq[b, h * SH:(h + 1) * SH, :].T)  # [D, SH]
        in_maps.append({
            "seqT": seqT_own.astype(_FP8),
            "wq": wq_f8,
            "wk": wk_f8,
            "wv": wvm_f8,
            "seqTh": seqT_own,
        })
    return in_maps


def _run(seq, Wq, Wk, Wv, mask, trace=False, **run_kwargs):
    nc = _get_nc()
    in_maps = _prep_in_maps(seq, Wq, Wk, Wv, mask)
    res = bass_utils.run_bass_kernel_spmd(
        nc, in_maps, core_ids=list(range(N_CORES)), trace=trace, **run_kwargs
    )
    out = np.empty((B, S, D), dtype=np.float32)
    for c in range(N_CORES):
        b, h = divmod(c, 2)
        out[b, h * SH:(h + 1) * SH, :] = res.results[c]["outT"].T
    return out, res


def kernel(seq, Wq, Wk, Wv, mask):
    out, _ = _run(seq, Wq, Wk, Wv, mask)
    return out


# revision 10
# speedup vs baseline: 1.5825x; 1.0156x over previous
"""Single-head full-attention layer on 8 Trainium2 NeuronCores (fp8 DoubleRow).

reference:
    q = seq @ Wq; k = seq @ Wk; v = seq @ Wv          # [B,S,D], D=1024
    scores = q @ k.T / sqrt(D)                        # [B,S,S]
    out = seq + softmax(scores) @ v * mask            # [B,S,D]

Sharding: 8 cores = 4 batches x 2 sequence-halves. Each core computes Q
for its own 1024 queries and K^T/V for its own 1024 keys; pairs exchange
K^T/V halves via 2-core AllGathers; each core then runs
softmax(QK^T)V + mask + residual for its query half.

All matmuls run in fp8(E4M3) with perf_mode=DoubleRow (two contraction
rows per PE cell): operands live in 3D SBUF tiles [128, ksub, free] and
each matmul consumes a [:, k:k+2, :] slice. Numerics:
  - host scales W by 32 before the fp8 cast (keeps N(0,1/1024) weights
    out of the fp8 subnormal range); the q/k factors cancel inside
    exp's scale (2^-15) and V's factor is folded into 1/colsum.
  - exp is shifted by -3 so attn values stay below fp8e4's +-240 max
    (softmax is shift-invariant).
  - scores/colsum/outT accumulate in fp32 PSUM; the normalization,
    output mask (folded into Wv on host) and fp32 residual are applied
    in the final fp32 stage.

The SPMD program is identical on all cores; the AllGather delivers keys
in global order for everyone.
"""

import numpy as np
import ml_dtypes

import concourse.bass as bass
import concourse.mybir as mybir
import concourse.tile as tile
from concourse import bacc, bass_utils

B, S, D = 4, 2048, 1024
N_CORES = 8
SH = S // 2          # queries / own keys per core
PD = 128             # partition dim
KD = D // PD         # 8 ksub chunks over d
KH = SH // PD        # 8 ksub chunks over own keys
KC = S // PD         # 16 ksub chunks over all keys
NT = 512             # matmul free-dim tile (one PSUM bank of fp32)
F8 = mybir.dt.float8e4
F32 = mybir.dt.float32
W_SCALE = 32.0
EXP_SCALE = 1.0 / (32.0 * W_SCALE * W_SCALE)   # 1/sqrt(D) / W_SCALE^2
EXP_SHIFT = -3.0
DR = mybir.MatmulPerfMode.DoubleRow

_FP8 = ml_dtypes.float8_e4m3
_GROUPS = [[0, 1], [2, 3], [4, 5], [6, 7]]


def _build_kernel(tc):
    nc = tc.nc
    seqT = nc.dram_tensor("seqT", [D, S], F8, kind="ExternalInput").ap()
    seqTq = nc.dram_tensor("seqTq", [D, SH], F8, kind="ExternalInput").ap()
    wq = nc.dram_tensor("wq", [D, D], F8, kind="ExternalInput").ap()
    wk = nc.dram_tensor("wk", [D, D], F8, kind="ExternalInput").ap()
    wv = nc.dram_tensor("wv", [D, D], F8, kind="ExternalInput").ap()
    seqTh = nc.dram_tensor("seqTh", [D, SH], F32, kind="ExternalInput").ap()
    outT = nc.dram_tensor("outT", [D, SH], F32, kind="ExternalOutput").ap()

    Exp = mybir.ActivationFunctionType.Exp

    with (
        tc.tile_pool(name="p_seq", bufs=1) as p_seq,
        tc.tile_pool(name="p_w", bufs=2) as p_w,
        tc.tile_pool(name="p_own", bufs=1) as p_own,
        tc.tile_pool(name="p_qt", bufs=1) as p_qt,
        tc.tile_pool(name="p_kt", bufs=1) as p_kt,
        tc.tile_pool(name="p_v", bufs=1) as p_v,
        tc.tile_pool(name="p_at", bufs=1) as p_at,
        tc.tile_pool(name="p_sh", bufs=2) as p_sh,
        tc.tile_pool(name="p_o", bufs=2) as p_o,
        tc.tile_pool(name="p_msc", bufs=1) as p_msc,
        tc.tile_pool(name="p_dram", bufs=1, space="DRAM") as p_dram,
        tc.tile_pool(name="p_mm", bufs=6, space="PSUM") as p_mm,
        tc.tile_pool(name="p_cs", bufs=1, space="PSUM") as p_cs,
    ):
        # ---- resident inputs ------------------------------------------------
        seq3 = p_seq.tile([PD, KD, S], F8, tag="seq", name="seq3")
        for j in range(KD):
            nc.sync.dma_start(seq3[:, j, :], seqT[j * PD:(j + 1) * PD, :])
        seqq3 = p_seq.tile([PD, KD, SH], F8, tag="seqq", name="seqq3")
        for j in range(KD):
            nc.sync.dma_start(seqq3[:, j, :], seqTq[j * PD:(j + 1) * PD, :])

        def load_w(w_dram, label):
            t = p_w.tile([PD, KD, D], F8, tag="w", name=label)
            for j in range(KD):
                nc.sync.dma_start(t[:, j, :], w_dram[j * PD:(j + 1) * PD, :])
            return t

        # collective bounce buffers (DRAM, Local) - V exchange only
        ib_v = p_dram.tile([SH, D], F8, tag="ibv", name="ib_v")
        ob_v = p_dram.tile([2, SH, D], F8, tag="obv", name="ob_v")

        # ---- KT = (seq @ Wk).T computed fully locally (cheaper in fp8 than
        # waiting on a K exchange: the cc barrier + transfer would idle PE) ---
        wk3 = load_w(wk, "wk3")
        kt3 = p_kt.tile([PD, KD, S], F8, tag="kt", name="kt3")
        for m in range(KD):
            for n in range(S // NT):
                ps = p_mm.tile([PD, NT], F32, tag="mm", name=f"ps_k{m}_{n}")
                for k in range(0, KD, 2):
                    nc.tensor.matmul(
                        ps[:],
                        wk3[:, k:k + 2, m * PD:(m + 1) * PD],
                        seq3[:, k:k + 2, n * NT:(n + 1) * NT],
                        start=(k == 0),
                        stop=(k == KD - 2),
                        perf_mode=DR,
                    )
                nc.vector.tensor_copy(kt3[:, m, n * NT:(n + 1) * NT], ps[:])

        # ---- V_own = seq_own @ (Wv * mask), bounce out, AllGather -----------
        wv3 = load_w(wv, "wv3")
        vo3 = p_own.tile([PD, KH, D], F8, tag="vo", name="vo3")
        for m in range(KH):
            for n in range(D // NT):
                ps = p_mm.tile([PD, NT], F32, tag="mm", name=f"ps_v{m}_{n}")
                for k in range(0, KD, 2):
                    nc.tensor.matmul(
                        ps[:],
                        seqq3[:, k:k + 2, m * PD:(m + 1) * PD],
                        wv3[:, k:k + 2, n * NT:(n + 1) * NT],
                        start=(k == 0),
                        stop=(k == KD - 2),
                        perf_mode=DR,
                    )
                nc.vector.tensor_copy(vo3[:, m, n * NT:(n + 1) * NT], ps[:])
            nc.sync.dma_start(ib_v[m * PD:(m + 1) * PD, :], vo3[:, m, :])
        nc.gpsimd.collective_compute(
            "AllGather", mybir.AluOpType.bypass, replica_groups=_GROUPS,
            ins=[ib_v.opt()], outs=[ob_v.opt()],
        )

        # ---- QT = (seq_own @ Wq).T (overlaps the collectives) ---------------
        wq3 = load_w(wq, "wq3")
        qt3 = p_qt.tile([PD, KD, SH], F8, tag="qt", name="qt3")
        for m in range(KD):
            for n in range(SH // NT):
                ps = p_mm.tile([PD, NT], F32, tag="mm", name=f"ps_q{m}_{n}")
                for k in range(0, KD, 2):
                    nc.tensor.matmul(
                        ps[:],
                        wq3[:, k:k + 2, m * PD:(m + 1) * PD],
                        seqq3[:, k:k + 2, n * NT:(n + 1) * NT],
                        start=(k == 0),
                        stop=(k == KD - 2),
                        perf_mode=DR,
                    )
                nc.vector.tensor_copy(qt3[:, m, n * NT:(n + 1) * NT], ps[:])

        # ---- gather exchanged V into SBUF -----------------------------------
        v3t = p_v.tile([PD, KC, D], F8, tag="v", name="v3t")
        for m in range(KC):
            r, mm_ = divmod(m, KH)
            nc.sync.dma_start(v3t[:, m, :], ob_v[r, mm_ * PD:(mm_ + 1) * PD, :])

        # ---- scoresT -> exp(shifted) -> colsum ------------------------------
        ones3 = p_msc.tile([PD, 2, 16], F8, tag="ones", name="ones3")
        nc.vector.memset(ones3[:], 1.0)
        ebias = p_msc.tile([PD, 1], F32, tag="ebias", name="ebias")
        nc.vector.memset(ebias[:], EXP_SHIFT)
        cs_ps = p_cs.tile([1, SH], F32, tag="cs", name="cs")
        at3 = p_at.tile([PD, KC, SH], F8, tag="at", name="at3")

        def colsum_mm(m):
            for n in range(SH // NT):
                nc.tensor.matmul(
                    cs_ps[:, n * NT:(n + 1) * NT],
                    ones3[:, 0:2, 0:1],
                    at3[:, m:m + 2, n * NT:(n + 1) * NT],
                    start=(m == 0),
                    stop=(m == KC - 2),
                    perf_mode=DR,
                )

        for m in range(KC):
            for n in range(SH // NT):
                ps = p_mm.tile([PD, NT], F32, tag="mm", name=f"ps_s{m}_{n}")
                for k in range(0, KD, 2):
                    nc.tensor.matmul(
                        ps[:],
                        kt3[:, k:k + 2, m * PD:(m + 1) * PD],
                        qt3[:, k:k + 2, n * NT:(n + 1) * NT],
                        start=(k == 0),
                        stop=(k == KD - 2),
                        perf_mode=DR,
                    )
                nc.scalar.activation(
                    at3[:, m, n * NT:(n + 1) * NT], ps[:], Exp,
                    bias=ebias[:], scale=EXP_SCALE,
                )
            # colsum pairs ksubs (m, m+1); emit one pair late so the PE
            # never waits on ACT's exp
            if m >= 3 and m % 2 == 1:
                colsum_mm(m - 3)
        colsum_mm(KC - 2)

        # ---- 1/colsum (including V's W_SCALE), broadcast across partitions --
        recip_sb = p_msc.tile([1, SH], F32, tag="recip", name="recip")
        nc.vector.reciprocal(recip_sb[:], cs_ps[:])
        nc.vector.tensor_scalar_mul(recip_sb[:], recip_sb[:], 1.0 / W_SCALE)
        bc_sb = p_msc.tile([PD, SH], F32, tag="bc", name="bc")
        nc.gpsimd.partition_broadcast(bc_sb[:], recip_sb[:])

        # ---- OT = V.T @ AT, normalize, add residual, store ------------------
        for m in range(KD):
            sh_t = p_sh.tile([PD, SH], F32, tag="sh", name=f"sh{m}")
            nc.sync.dma_start(sh_t[:], seqTh[m * PD:(m + 1) * PD, :])
            o_t = p_o.tile([PD, SH], F32, tag="o", name=f"o{m}")
            for n in range(SH // NT):
                ps = p_mm.tile([PD, NT], F32, tag="mm", name=f"ps_o{m}_{n}")
                for k in range(0, KC, 2):
                    nc.tensor.matmul(
                        ps[:],
                        v3t[:, k:k + 2, m * PD:(m + 1) * PD],
                        at3[:, k:k + 2, n * NT:(n + 1) * NT],
                        start=(k == 0),
                        stop=(k == KC - 2),
                        perf_mode=DR,
                    )
                nc.vector.tensor_mul(
                    o_t[:, n * NT:(n + 1) * NT], ps[:],
                    bc_sb[:, n * NT:(n + 1) * NT],
                )
            eng = nc.vector if m % 2 == 0 else nc.gpsimd
            eng.tensor_add(o_t[:], o_t[:], sh_t[:])
            nc.sync.dma_start(outT[m * PD:(m + 1) * PD, :], o_t[:])


_NC_CACHE = None


def _get_nc():
    global _NC_CACHE
    if _NC_CACHE is None:
        nc = bacc.Bacc(
            "TRN2", target_bir_lowering=False, debug=False, num_devices=N_CORES
        )
        with tile.TileContext(nc) as tc:
            _build_kernel(tc)
        nc.compile()
        _NC_CACHE = nc
    return _NC_CACHE


def _prep_in_maps(seq, Wq, Wk, Wv, mask):
    seq = np.asarray(seq, dtype=np.float32)
    wq_f8 = (np.asarray(Wq, dtype=np.float32) * W_SCALE).astype(_FP8)
    wk_f8 = (np.asarray(Wk, dtype=np.float32) * W_SCALE).astype(_FP8)
    wvm_f8 = (np.asarray(Wv, dtype=np.float32)
              * np.asarray(mask, dtype=np.float32)[None, :] * W_SCALE).astype(_FP8)
    in_maps = []
    seqT_nat = {}
    for b in range(B):
        seqT_nat[b] = np.ascontiguousarray(seq[b].T).astype(_FP8)  # [D, S]
    for c in range(N_CORES):
        b, h = divmod(c, 2)
        seqT_own = np.ascontiguousarray(seq[b, h * SH:(h + 1) * SH, :].T)  # [D, SH]
        in_maps.append({
            "seqT": seqT_nat[b],
            "seqTq": seqT_own.astype(_FP8),
            "wq": wq_f8,
            "wk": wk_f8,
            "wv": wvm_f8,
            "seqTh": seqT_own,
        })
    return in_maps


def _run(seq, Wq, Wk, Wv, mask, trace=False, **run_kwargs):
    nc = _get_nc()
    in_maps = _prep_in_maps(seq, Wq, Wk, Wv, mask)
    res = bass_utils.run_bass_kernel_spmd(
        nc, in_maps, core_ids=list(range(N_CORES)), trace=trace, **run_kwargs
    )
    out = np.empty((B, S, D), dtype=np.float32)
    for c in range(N_CORES):
        b, h = divmod(c, 2)
        out[b, h * SH:(h + 1) * SH, :] = res.results[c]["outT"].T
    return out, res


def kernel(seq, Wq, Wk, Wv, mask):
    out, _ = _run(seq, Wq, Wk, Wv, mask)
    return out
